# revision 18
# baseline (speedup 1.0000x reference)
"""Trainium2 Bass kernel for nn_AttentionPool (gnn_message_passing).

Strategy (v3: oct-grouped pooling, host onehot masks)
-----------------------------------------------------
Math restructure (equivalent to the reference up to fp rounding):
  score[n,h] = context_h[n,:] @ V[:,h] + c[h]      (fold W_lin/W_att/b_lin/b_att)
  p = exp(leaky_relu(score, 0.2))                  (skip segment-max: scores are
                                                    O(1); softmax is shift-invariant)
  denom[cls,h]   = sum_{n: y=cls} p[n,h]
  pooled[cls,h,:] = sum_{n: y=cls} p[n,h]*context_h[n,:] / denom[cls,h]

Sharding: BY CLASS. Core k owns classes [125k, 125k+125). Within a core,
classes go into 16 "octs" of 8 classes, each with a fixed capacity of
8 tiles (1024 node slots vs ~800 expected fill). Per tile j of oct o:
  o_w[n, c*4+h] = mask8[n,c] * p[n,h]        (mask8 = host-built onehot of
                                              class-within-oct; ONE fused
                                              broadcast tensor_tensor builds
                                              o_w for a whole 16-tile supertile)
  PSUM[0:32, 0:257] += o_w^T @ [H | 1]       (one 257-wide matmul per tile,
                                              8-tile accumulation per oct)
Supertile = 2 octs = 16 tiles; the two octs use different PSUM banks.
Scores are computed 16 tiles at a time exactly as in the proven baseline.
All DMA reads are partition-contiguous host layouts.
"""

import sys

sys.path.insert(0, "/opt/trn_rl_repo")

import numpy as np
import ml_dtypes

BF = ml_dtypes.bfloat16

N = 100000
INC = 256
NHEAD = 4
OUTC = 128
NCLS = 1000
NCORES = 8
CPC = NCLS // NCORES  # 125 classes per core
CPO = 8  # classes per oct
NOCT = 16  # octs per core (16*8 = 128 >= 125)
MW = INC + 1  # pool moving width: 256 feats + ones col
HW = 258  # stored row width (257 + 1 pad -> 516B rows, 4B aligned)
MCOL = NHEAD * CPO  # 32 stationary columns (c-major, h-minor)
F8 = ml_dtypes.float8_e4m3  # TRN float8e4-compatible (bias 7, +-240 max)

_PROG_CACHE = {}
LAST_RESULT = None
LAST_PROFILE = None


def build_program(o_tiles=8):
    """Build + compile the SPMD Bass program. o_tiles = tiles per oct."""
    from concourse import bacc, mybir, tile

    f32 = mybir.dt.float32
    bf16 = mybir.dt.bfloat16
    fp8 = mybir.dt.float8e4
    AF = mybir.ActivationFunctionType
    OP = mybir.AluOpType

    t_tiles = NOCT * o_tiles
    cap = t_tiles * 128
    SUP = 2 * o_tiles  # tiles per supertile (2 octs)
    NSUP = NOCT // 2

    nc = bacc.Bacc(
        "TRN2", target_bir_lowering=False, debug=False, num_devices=NCORES
    )

    # one packed stream tensor; per-supertile slab = [ht | hn | mk]:
    #   ht[c*SUP*128 + j*128 + n] = h[slot, c*128+p]   (transposed copy)
    #   hn[j*HW + f] = h row | ones | pad              (node-major copy)
    #   mk[j*CPO + c] = onehot(class-within-oct)
    STW = SUP * (256 + HW + CPO)
    stream = nc.dram_tensor(
        "stream", [128, NSUP * STW], bf16, kind="ExternalInput"
    ).ap()
    # packed consts: col 0 = W_att, cols 1:5 = b_lin_r, col 5 = b_att (bcast),
    # cols 6: = wl_r ([o, h*256+k] = W_lin[h*128+o, k])
    consts = nc.dram_tensor(
        "consts", [128, 6 + NHEAD * INC], f32, kind="ExternalInput"
    ).ap()
    out = nc.dram_tensor("out_pool", [MCOL, NOCT * MW], f32, kind="ExternalOutput").ap()

    with tile.TileContext(nc) as tc:
        with (
            tc.tile_pool(name="const", bufs=1) as cpool,
            tc.tile_pool(name="stream", bufs=3) as sb,
            tc.tile_pool(name="work", bufs=3) as sg,
            tc.tile_pool(name="ow", bufs=3) as owp,
            tc.tile_pool(name="ps", bufs=2, space="PSUM") as ps,
            tc.tile_pool(name="acc", bufs=4, space="PSUM") as accp,
        ):
            # ---- stream loads (emitted first so the DMA queue starts on the
            # critical data while the V-fold prologue runs) -------------------
            stream_tiles = {}
            p_tiles = {}
            ow_tiles = {}

            def load(s_):
                st = sb.tile([128, STW], bf16, tag="st")
                ht = st[:, 0 : SUP * 256]
                hn = st[:, SUP * 256 : SUP * (256 + HW)]
                mk = st[:, SUP * (256 + HW) : STW]
                o0 = s_ * STW
                nc.sync.dma_start(out=ht, in_=stream[:, o0 : o0 + SUP * 256])
                nc.scalar.dma_start(
                    out=hn,
                    in_=stream[:, o0 + SUP * 256 : o0 + SUP * (256 + HW)],
                )
                nc.sync.dma_start(
                    out=mk, in_=stream[:, o0 + SUP * (256 + HW) : o0 + STW]
                )
                stream_tiles[s_] = (ht, hn, mk)

            # ---- constants (one tiny DMA first: the V-fold gates all PE work)
            cst = cpool.tile([128, 6 + NHEAD * INC], f32)
            nc.sync.dma_start(out=cst[:], in_=consts)
            watt_sb = cst[:, 0:1]
            blin_sb = cst[:, 1 : 1 + NHEAD]
            batt_sb = cst[:1, 5:6]
            wl_all = cst[:, 6:]
            ones_sb = cpool.tile([1, 128], bf16)
            nc.vector.memset(ones_sb[:1], 1.0)

            load(0)
            load(1)

            # ---- fold W_lin/W_att into V [256,4] (two 128-chunks), c [1,4] --
            v_bf = []
            for ch in range(2):
                v_ps = ps.tile([128, NHEAD], f32, tag="sps", padded_shape=[128, 512])
                for h in range(NHEAD):
                    nc.tensor.matmul(
                        v_ps[:, h : h + 1],
                        lhsT=wl_all[:, h * INC + ch * 128 : h * INC + ch * 128 + 128],
                        rhs=watt_sb[:],
                        start=True, stop=True,
                    )
                vb = cpool.tile([128, NHEAD], bf16, tag=f"vbf{ch}")
                nc.vector.tensor_copy(out=vb[:], in_=v_ps[:])
                v_bf.append(vb)

            c_ps = ps.tile([1, NHEAD], f32, tag="sps", padded_shape=[1, 512])
            nc.tensor.matmul(c_ps[:1], lhsT=watt_sb[:], rhs=blin_sb[:],
                             start=True, stop=True)
            c_bf = cpool.tile([1, NHEAD], bf16)
            nc.scalar.activation(c_bf[:1], c_ps[:1], AF.Identity,
                                 bias=batt_sb[:1, 0:1])
            c_rep = cpool.tile([1, SUP * NHEAD], bf16)
            for r in range(SUP):
                nc.vector.tensor_copy(
                    out=c_rep[:1, r * NHEAD : (r + 1) * NHEAD], in_=c_bf[:1, :]
                )

            out_sb = cpool.tile([128, NOCT * MW], f32)

            def scores(s_):
                ht, hn, mk = stream_tiles[s_]
                s_ps = ps.tile(
                    [128, SUP * NHEAD], f32, tag="sps", padded_shape=[128, 512]
                )
                nw = SUP * NHEAD
                nc.tensor.matmul(
                    s_ps[:, :nw], lhsT=ones_sb[:1], rhs=c_rep[:1, :nw],
                    start=True, stop=False, skip_group_check=True,
                )
                for j in range(SUP):
                    sl = slice(j * NHEAD, (j + 1) * NHEAD)
                    nc.tensor.matmul(
                        s_ps[:, sl], lhsT=ht[:, j * 128 : (j + 1) * 128],
                        rhs=v_bf[0][:], start=False, stop=False,
                        skip_group_check=True,
                    )
                    nc.tensor.matmul(
                        s_ps[:, sl],
                        lhsT=ht[:, SUP * 128 + j * 128 : SUP * 128 + (j + 1) * 128],
                        rhs=v_bf[1][:], start=False, stop=(j == SUP - 1),
                        skip_group_check=True,
                    )
                # p = exp(leaky_relu(s)); leaky(x) = max(x, 0.2x)
                t02 = sg.tile([128, SUP * NHEAD], f32, tag="t02")
                nc.vector.tensor_scalar_mul(t02[:, :nw], s_ps[:, :nw], 0.2)
                slr = sg.tile([128, SUP * NHEAD], f32, tag="slr")
                nc.vector.tensor_tensor(
                    out=slr[:, :nw], in0=s_ps[:, :nw], in1=t02[:, :nw], op=OP.max
                )
                p_sb = sg.tile([128, SUP * NHEAD], f32, tag="p")
                nc.scalar.activation(p_sb[:, :nw], slr[:, :nw], AF.Exp)
                p_tiles[s_] = p_sb

                # one fused broadcast multiply builds o_w for all 16 tiles:
                # o_w[n, j*32 + c*4 + h] = mask8[n, j*8+c] * p[n, j*4+h]
                o_w = owp.tile([128, SUP * MCOL], bf16, tag="ow")
                o4 = o_w[:].rearrange("p (j c h) -> p j c h", c=CPO, h=NHEAD)
                p4 = (
                    p_sb[:, : SUP * NHEAD]
                    .rearrange("p (j h) -> p j h", h=NHEAD)
                    .unsqueeze(2)
                    .broadcast_to([128, SUP, CPO, NHEAD])
                )
                m4 = (
                    mk[:, : SUP * CPO]
                    .rearrange("p (j c) -> p j c", c=CPO)
                    .unsqueeze(3)
                    .broadcast_to([128, SUP, CPO, NHEAD])
                )
                nc.vector.tensor_tensor(out=o4, in0=m4, in1=p4, op=OP.mult)
                ow_tiles[s_] = o_w

            def pools(s_):
                ht, hn, mk = stream_tiles.pop(s_)
                p_tiles.pop(s_)
                o_w = ow_tiles.pop(s_)
                for half in range(2):
                    oct_ = 2 * s_ + half
                    pool_ps = accp.tile(
                        [MCOL, MW], f32, tag="pool", padded_shape=[MCOL, 512]
                    )
                    for jj in range(o_tiles):
                        j = half * o_tiles + jj
                        nc.tensor.matmul(
                            pool_ps[0:MCOL, 0:MW],
                            lhsT=o_w[:, j * MCOL : (j + 1) * MCOL],
                            rhs=hn[:, j * HW : j * HW + MW],
                            start=(jj == 0), stop=(jj == o_tiles - 1),
                        )
                    nc.scalar.activation(
                        out_sb[0:MCOL, oct_ * MW : (oct_ + 1) * MW],
                        pool_ps[0:MCOL, 0:MW], AF.Copy,
                    )
                    # drain finished oct pairs to HBM as we go
                    if oct_ % 2 == 1:
                        g = oct_ // 2
                        nc.sync.dma_start(
                            out=out[:, g * 2 * MW : (g + 1) * 2 * MW],
                            in_=out_sb[0:MCOL, g * 2 * MW : (g + 1) * 2 * MW],
                        )

            scores(0)
            for s_ in range(NSUP):
                if s_ + 2 < NSUP:
                    load(s_ + 2)
                if s_ + 1 < NSUP:
                    scores(s_ + 1)
                pools(s_)

    nc.compile()
    return nc


def _prep_inputs(context_h, W_lin, b_lin, W_att, b_att, context_y, o_tiles):
    """Host-side shard: bucket nodes into 16 fixed-capacity octs per core,
    partition-contiguous layouts. Pure data movement + dtype casts."""
    h = np.ascontiguousarray(np.asarray(context_h, dtype=np.float32))
    y = np.asarray(context_y).astype(np.int64)
    order = np.argsort(y, kind="stable")
    ys = y[order]

    t_tiles = NOCT * o_tiles
    cap = t_tiles * 128
    oslots = o_tiles * 128
    SUP = 2 * o_tiles
    NSUP = NOCT // 2

    W_lin = np.asarray(W_lin, dtype=np.float32)
    consts = np.zeros((128, 6 + NHEAD * INC), dtype=np.float32)
    consts[:, 0] = np.asarray(W_att, dtype=np.float32)
    consts[:, 1 : 1 + NHEAD] = (
        np.asarray(b_lin, dtype=np.float32).reshape(NHEAD, OUTC).T
    )
    consts[:, 5] = float(np.asarray(b_att, dtype=np.float32).reshape(-1)[0])
    consts[:, 6:] = W_lin.reshape(NHEAD, OUTC, INC).transpose(1, 0, 2).reshape(
        OUTC, NHEAD * INC
    )
    consts = np.ascontiguousarray(consts)

    # oct boundaries: core k, oct o covers classes [125k+8o, min(125k+8o+8, ...))
    cls_edges = []
    for k in range(NCORES):
        for o in range(NOCT):
            cls_edges.append(k * CPC + min(CPC, o * CPO))
    cls_edges.append(NCLS)
    bounds = np.searchsorted(ys, np.array(cls_edges))

    in_maps = []
    for k in range(NCORES):
        slots = np.full(cap, -1, dtype=np.int64)
        yo_flat = np.full(cap, -1.0, dtype=np.float32)  # class-within-oct
        for o in range(NOCT):
            lo, hi = bounds[k * NOCT + o], bounds[k * NOCT + o + 1]
            cnt = hi - lo
            assert cnt <= oslots, (k, o, cnt, oslots)
            slots[o * oslots : o * oslots + cnt] = order[lo:hi]
            yo_flat[o * oslots : o * oslots + cnt] = (
                ys[lo:hi] - k * CPC - o * CPO
            ).astype(np.float32)

        valid = slots >= 0
        hsel = np.zeros((cap, INC), dtype=np.float32)
        hsel[valid] = h[slots[valid]]
        hsel_bf = hsel.astype(BF)

        h_nm = np.zeros((t_tiles, 128, HW), dtype=BF)
        h_nm[:, :, :INC] = hsel_bf.reshape(t_tiles, 128, INC)
        h_nm[:, :, INC] = BF(1.0)
        h_nm = h_nm.transpose(1, 0, 2).reshape(128, NSUP, SUP * HW)
        # [p, s, c, m] = hsel[s*SUP*128 + m, c*128 + p]
        h_tr = (
            hsel_bf
            .reshape(NSUP, SUP * 128, 2, 128)
            .transpose(3, 0, 2, 1)
            .reshape(128, NSUP, SUP * 256)
        )
        m8 = (
            yo_flat[:, None] == np.arange(CPO, dtype=np.float32)[None, :]
        ).astype(BF)
        m8 = m8.reshape(t_tiles, 128, CPO).transpose(1, 0, 2).reshape(
            128, NSUP, SUP * CPO
        )
        stream = np.ascontiguousarray(
            np.concatenate([h_tr, h_nm, m8], axis=2).reshape(128, -1)
        )
        in_maps.append(
            {
                "stream": stream,
                "consts": consts,
            }
        )
    return in_maps


def _unshard(res):
    outp = np.empty((NCLS, NHEAD * INC), dtype=np.float32)
    for k in range(NCORES):
        o = np.asarray(res.results[k]["out_pool"])  # [32, 16*257]
        blk = o.reshape(MCOL, NOCT, MW).transpose(1, 0, 2)  # [16, 32, 257]
        blk = blk.reshape(NOCT, CPO, NHEAD, MW)
        blk = blk.reshape(NOCT * CPO, NHEAD, MW)[:CPC]  # [125, 4, 257]
        pool = blk[:, :, :INC]
        den = blk[:, :, INC]
        den = np.where(den != 0.0, den, 1.0)
        outp[k * CPC : (k + 1) * CPC] = (pool / den[:, :, None]).reshape(
            CPC, NHEAD * INC
        )
    return outp


def kernel(context_h, W_lin, b_lin, W_att, b_att, context_y, num_classes):
    global LAST_RESULT, LAST_PROFILE
    import os

    assert int(num_classes) == NCLS

    from concourse.bass_utils import run_bass_kernel_spmd

    # fixed oct capacity; bump tiles-per-oct only if the data demands it
    y = np.asarray(context_y).astype(np.int64)
    cnt = np.bincount(y, minlength=NCLS)
    gmax = 0
    for k in range(NCORES):
        c = cnt[k * CPC : (k + 1) * CPC]
        for o in range(NOCT):
            gmax = max(gmax, int(c[o * CPO : min(CPC, (o + 1) * CPO)].sum()))
    o_tiles = max(1, -(-gmax // 128))

    in_maps = _prep_inputs(
        context_h, W_lin, b_lin, W_att, b_att, context_y, o_tiles
    )
    if o_tiles not in _PROG_CACHE:
        _PROG_CACHE[o_tiles] = build_program(o_tiles=o_tiles)
    nc = _PROG_CACHE[o_tiles]
    core_ids = list(range(NCORES))
    res = run_bass_kernel_spmd(nc, in_maps, core_ids)
    LAST_RESULT = res
    outp = _unshard(res)  # materialize before any profiled re-run

    if os.environ.get("KERNEL_PROFILE") == "1":
        LAST_PROFILE = run_bass_kernel_spmd(nc, in_maps, core_ids, trace=True)

    return outp


# revision 19
# speedup vs baseline: 1.1378x; 1.1378x over previous
"""Trainium2 Bass kernel for nn_AttentionPool (gnn_message_passing).

Strategy (v3: oct-grouped pooling, host onehot masks)
-----------------------------------------------------
Math restructure (equivalent to the reference up to fp rounding):
  score[n,h] = context_h[n,:] @ V[:,h] + c[h]      (fold W_lin/W_att/b_lin/b_att)
  p = exp(leaky_relu(score, 0.2))                  (skip segment-max: scores are
                                                    O(1); softmax is shift-invariant)
  denom[cls,h]   = sum_{n: y=cls} p[n,h]
  pooled[cls,h,:] = sum_{n: y=cls} p[n,h]*context_h[n,:] / denom[cls,h]

Sharding: BY CLASS. Core k owns classes [125k, 125k+125). Within a core,
classes go into 16 "octs" of 8 classes, each with a fixed capacity of
8 tiles (1024 node slots vs ~800 expected fill). Per tile j of oct o:
  o_w[n, c*4+h] = mask8[n,c] * p[n,h]        (mask8 = host-built onehot of
                                              class-within-oct; ONE fused
                                              broadcast tensor_tensor builds
                                              o_w for a whole 16-tile supertile)
  PSUM[0:32, 0:257] += o_w^T @ [H | 1]       (one 257-wide matmul per tile,
                                              8-tile accumulation per oct)
Supertile = 2 octs = 16 tiles; the two octs use different PSUM banks.
Scores are computed 16 tiles at a time exactly as in the proven baseline.
All DMA reads are partition-contiguous host layouts.
"""

import sys

sys.path.insert(0, "/opt/trn_rl_repo")

import numpy as np
import ml_dtypes

BF = ml_dtypes.bfloat16

N = 100000
INC = 256
NHEAD = 4
OUTC = 128
NCLS = 1000
NCORES = 8
CPC = NCLS // NCORES  # 125 classes per core
CPO = 8  # classes per oct
NOCT = 16  # octs per core (16*8 = 128 >= 125)
MW = INC + 1  # pool moving width: 256 feats + ones col
HW = 258  # stored row width (257 + 1 pad -> 516B rows, 4B aligned)
MCOL = NHEAD * CPO  # 32 stationary columns (c-major, h-minor)
F8 = ml_dtypes.float8_e4m3  # TRN float8e4-compatible (bias 7, +-240 max)

_PROG_CACHE = {}
LAST_RESULT = None
LAST_PROFILE = None


def build_program(o_tiles=8):
    """Build + compile the SPMD Bass program. o_tiles = tiles per oct."""
    from concourse import bacc, mybir, tile

    f32 = mybir.dt.float32
    bf16 = mybir.dt.bfloat16
    fp8 = mybir.dt.float8e4
    AF = mybir.ActivationFunctionType
    OP = mybir.AluOpType

    t_tiles = NOCT * o_tiles
    cap = t_tiles * 128
    SUP = 2 * o_tiles  # tiles per supertile (2 octs)
    NSUP = NOCT // 2

    nc = bacc.Bacc(
        "TRN2", target_bir_lowering=False, debug=False, num_devices=NCORES
    )

    # one packed stream tensor; per-supertile slab = [ht | hn | mk]:
    #   ht[c*SUP*128 + j*128 + n] = h[slot, c*128+p]   (transposed copy)
    #   hn[j*HW + f] = h row | ones | pad              (node-major copy)
    #   mk[j*CPO + c] = onehot(class-within-oct)
    STW = SUP * (256 + HW + CPO)
    stream = nc.dram_tensor(
        "stream", [128, NSUP * STW], bf16, kind="ExternalInput"
    ).ap()
    # packed consts: col 0 = W_att, cols 1:5 = b_lin_r, col 5 = b_att (bcast),
    # cols 6: = wl_r ([o, h*256+k] = W_lin[h*128+o, k])
    consts = nc.dram_tensor(
        "consts", [128, 6 + NHEAD * INC], f32, kind="ExternalInput"
    ).ap()
    out = nc.dram_tensor("out_pool", [MCOL, NOCT * MW], f32, kind="ExternalOutput").ap()

    with tile.TileContext(nc) as tc:
        with (
            tc.tile_pool(name="const", bufs=1) as cpool,
            tc.tile_pool(name="stream", bufs=3) as sb,
            tc.tile_pool(name="work", bufs=3) as sg,
            tc.tile_pool(name="ow", bufs=3) as owp,
            tc.tile_pool(name="ps", bufs=2, space="PSUM") as ps,
            tc.tile_pool(name="acc", bufs=4, space="PSUM") as accp,
        ):
            # ---- stream loads (emitted first so the DMA queue starts on the
            # critical data while the V-fold prologue runs) -------------------
            stream_tiles = {}
            p_tiles = {}
            ow_tiles = {}

            def load(s_):
                st = sb.tile([128, STW], bf16, tag="st")
                ht = st[:, 0 : SUP * 256]
                hn = st[:, SUP * 256 : SUP * (256 + HW)]
                mk = st[:, SUP * (256 + HW) : STW]
                o0 = s_ * STW
                nc.sync.dma_start(out=ht, in_=stream[:, o0 : o0 + SUP * 256])
                nc.sync.dma_start(
                    out=hn,
                    in_=stream[:, o0 + SUP * 256 : o0 + SUP * (256 + HW)],
                )
                nc.sync.dma_start(
                    out=mk, in_=stream[:, o0 + SUP * (256 + HW) : o0 + STW]
                )
                stream_tiles[s_] = (ht, hn, mk)

            # ---- constants (one tiny DMA first: the V-fold gates all PE work)
            cst = cpool.tile([128, 6 + NHEAD * INC], f32)
            nc.sync.dma_start(out=cst[:], in_=consts)
            watt_sb = cst[:, 0:1]
            blin_sb = cst[:, 1 : 1 + NHEAD]
            batt_sb = cst[:1, 5:6]
            wl_all = cst[:, 6:]
            ones_sb = cpool.tile([1, 128], bf16)
            nc.vector.memset(ones_sb[:1], 1.0)

            load(0)
            load(1)

            # ---- fold W_lin/W_att into V [256,4] (two 128-chunks), c [1,4] --
            v_bf = []
            for ch in range(2):
                v_ps = ps.tile([128, NHEAD], f32, tag="sps", padded_shape=[128, 512])
                for h in range(NHEAD):
                    nc.tensor.matmul(
                        v_ps[:, h : h + 1],
                        lhsT=wl_all[:, h * INC + ch * 128 : h * INC + ch * 128 + 128],
                        rhs=watt_sb[:],
                        start=True, stop=True,
                    )
                vb = cpool.tile([128, NHEAD], bf16, tag=f"vbf{ch}")
                nc.vector.tensor_copy(out=vb[:], in_=v_ps[:])
                v_bf.append(vb)

            c_ps = ps.tile([1, NHEAD], f32, tag="sps", padded_shape=[1, 512])
            nc.tensor.matmul(c_ps[:1], lhsT=watt_sb[:], rhs=blin_sb[:],
                             start=True, stop=True)
            c_bf = cpool.tile([1, NHEAD], bf16)
            nc.scalar.activation(c_bf[:1], c_ps[:1], AF.Identity,
                                 bias=batt_sb[:1, 0:1])
            c_rep = cpool.tile([1, SUP * NHEAD], bf16)
            for r in range(SUP):
                nc.vector.tensor_copy(
                    out=c_rep[:1, r * NHEAD : (r + 1) * NHEAD], in_=c_bf[:1, :]
                )

            out_sb = cpool.tile([128, NOCT * MW], f32)

            def scores(s_):
                ht, hn, mk = stream_tiles[s_]
                s_ps = ps.tile(
                    [128, SUP * NHEAD], f32, tag="sps", padded_shape=[128, 512]
                )
                nw = SUP * NHEAD
                nc.tensor.matmul(
                    s_ps[:, :nw], lhsT=ones_sb[:1], rhs=c_rep[:1, :nw],
                    start=True, stop=False, skip_group_check=True,
                )
                for j in range(SUP):
                    sl = slice(j * NHEAD, (j + 1) * NHEAD)
                    nc.tensor.matmul(
                        s_ps[:, sl], lhsT=ht[:, j * 128 : (j + 1) * 128],
                        rhs=v_bf[0][:], start=False, stop=False,
                        skip_group_check=True,
                    )
                    nc.tensor.matmul(
                        s_ps[:, sl],
                        lhsT=ht[:, SUP * 128 + j * 128 : SUP * 128 + (j + 1) * 128],
                        rhs=v_bf[1][:], start=False, stop=(j == SUP - 1),
                        skip_group_check=True,
                    )
                # p = exp(leaky_relu(s)); leaky(x) = max(x, 0.2x)
                t02 = sg.tile([128, SUP * NHEAD], f32, tag="t02")
                nc.vector.tensor_scalar_mul(t02[:, :nw], s_ps[:, :nw], 0.2)
                slr = sg.tile([128, SUP * NHEAD], f32, tag="slr")
                nc.vector.tensor_tensor(
                    out=slr[:, :nw], in0=s_ps[:, :nw], in1=t02[:, :nw], op=OP.max
                )
                p_sb = sg.tile([128, SUP * NHEAD], f32, tag="p")
                nc.scalar.activation(p_sb[:, :nw], slr[:, :nw], AF.Exp)
                p_tiles[s_] = p_sb

                # one fused broadcast multiply builds o_w for all 16 tiles:
                # o_w[n, j*32 + c*4 + h] = mask8[n, j*8+c] * p[n, j*4+h]
                o_w = owp.tile([128, SUP * MCOL], bf16, tag="ow")
                o4 = o_w[:].rearrange("p (j c h) -> p j c h", c=CPO, h=NHEAD)
                p4 = (
                    p_sb[:, : SUP * NHEAD]
                    .rearrange("p (j h) -> p j h", h=NHEAD)
                    .unsqueeze(2)
                    .broadcast_to([128, SUP, CPO, NHEAD])
                )
                m4 = (
                    mk[:, : SUP * CPO]
                    .rearrange("p (j c) -> p j c", c=CPO)
                    .unsqueeze(3)
                    .broadcast_to([128, SUP, CPO, NHEAD])
                )
                nc.vector.tensor_tensor(out=o4, in0=m4, in1=p4, op=OP.mult)
                ow_tiles[s_] = o_w

            def pools(s_):
                ht, hn, mk = stream_tiles.pop(s_)
                p_tiles.pop(s_)
                o_w = ow_tiles.pop(s_)
                for half in range(2):
                    oct_ = 2 * s_ + half
                    pool_ps = accp.tile(
                        [MCOL, MW], f32, tag="pool", padded_shape=[MCOL, 512]
                    )
                    for jj in range(o_tiles):
                        j = half * o_tiles + jj
                        nc.tensor.matmul(
                            pool_ps[0:MCOL, 0:MW],
                            lhsT=o_w[:, j * MCOL : (j + 1) * MCOL],
                            rhs=hn[:, j * HW : j * HW + MW],
                            start=(jj == 0), stop=(jj == o_tiles - 1),
                        )
                    nc.scalar.activation(
                        out_sb[0:MCOL, oct_ * MW : (oct_ + 1) * MW],
                        pool_ps[0:MCOL, 0:MW], AF.Copy,
                    )
                    # drain finished oct pairs to HBM as we go
                    if oct_ % 2 == 1:
                        g = oct_ // 2
                        nc.sync.dma_start(
                            out=out[:, g * 2 * MW : (g + 1) * 2 * MW],
                            in_=out_sb[0:MCOL, g * 2 * MW : (g + 1) * 2 * MW],
                        )

            scores(0)
            for s_ in range(NSUP):
                if s_ + 2 < NSUP:
                    load(s_ + 2)
                if s_ + 1 < NSUP:
                    scores(s_ + 1)
                pools(s_)

    nc.compile()
    return nc


def _prep_inputs(context_h, W_lin, b_lin, W_att, b_att, context_y, o_tiles):
    """Host-side shard: bucket nodes into 16 fixed-capacity octs per core,
    partition-contiguous layouts. Pure data movement + dtype casts."""
    h = np.ascontiguousarray(np.asarray(context_h, dtype=np.float32))
    y = np.asarray(context_y).astype(np.int64)
    order = np.argsort(y, kind="stable")
    ys = y[order]

    t_tiles = NOCT * o_tiles
    cap = t_tiles * 128
    oslots = o_tiles * 128
    SUP = 2 * o_tiles
    NSUP = NOCT // 2

    W_lin = np.asarray(W_lin, dtype=np.float32)
    consts = np.zeros((128, 6 + NHEAD * INC), dtype=np.float32)
    consts[:, 0] = np.asarray(W_att, dtype=np.float32)
    consts[:, 1 : 1 + NHEAD] = (
        np.asarray(b_lin, dtype=np.float32).reshape(NHEAD, OUTC).T
    )
    consts[:, 5] = float(np.asarray(b_att, dtype=np.float32).reshape(-1)[0])
    consts[:, 6:] = W_lin.reshape(NHEAD, OUTC, INC).transpose(1, 0, 2).reshape(
        OUTC, NHEAD * INC
    )
    consts = np.ascontiguousarray(consts)

    # oct boundaries: core k, oct o covers classes [125k+8o, min(125k+8o+8, ...))
    cls_edges = []
    for k in range(NCORES):
        for o in range(NOCT):
            cls_edges.append(k * CPC + min(CPC, o * CPO))
    cls_edges.append(NCLS)
    bounds = np.searchsorted(ys, np.array(cls_edges))

    in_maps = []
    for k in range(NCORES):
        slots = np.full(cap, -1, dtype=np.int64)
        yo_flat = np.full(cap, -1.0, dtype=np.float32)  # class-within-oct
        for o in range(NOCT):
            lo, hi = bounds[k * NOCT + o], bounds[k * NOCT + o + 1]
            cnt = hi - lo
            assert cnt <= oslots, (k, o, cnt, oslots)
            slots[o * oslots : o * oslots + cnt] = order[lo:hi]
            yo_flat[o * oslots : o * oslots + cnt] = (
                ys[lo:hi] - k * CPC - o * CPO
            ).astype(np.float32)

        valid = slots >= 0
        hsel = np.zeros((cap, INC), dtype=np.float32)
        hsel[valid] = h[slots[valid]]
        hsel_bf = hsel.astype(BF)

        h_nm = np.zeros((t_tiles, 128, HW), dtype=BF)
        h_nm[:, :, :INC] = hsel_bf.reshape(t_tiles, 128, INC)
        h_nm[:, :, INC] = BF(1.0)
        h_nm = h_nm.transpose(1, 0, 2).reshape(128, NSUP, SUP * HW)
        # [p, s, c, m] = hsel[s*SUP*128 + m, c*128 + p]
        h_tr = (
            hsel_bf
            .reshape(NSUP, SUP * 128, 2, 128)
            .transpose(3, 0, 2, 1)
            .reshape(128, NSUP, SUP * 256)
        )
        m8 = (
            yo_flat[:, None] == np.arange(CPO, dtype=np.float32)[None, :]
        ).astype(BF)
        m8 = m8.reshape(t_tiles, 128, CPO).transpose(1, 0, 2).reshape(
            128, NSUP, SUP * CPO
        )
        stream = np.ascontiguousarray(
            np.concatenate([h_tr, h_nm, m8], axis=2).reshape(128, -1)
        )
        in_maps.append(
            {
                "stream": stream,
                "consts": consts,
            }
        )
    return in_maps


def _unshard(res):
    outp = np.empty((NCLS, NHEAD * INC), dtype=np.float32)
    for k in range(NCORES):
        o = np.asarray(res.results[k]["out_pool"])  # [32, 16*257]
        blk = o.reshape(MCOL, NOCT, MW).transpose(1, 0, 2)  # [16, 32, 257]
        blk = blk.reshape(NOCT, CPO, NHEAD, MW)
        blk = blk.reshape(NOCT * CPO, NHEAD, MW)[:CPC]  # [125, 4, 257]
        pool = blk[:, :, :INC]
        den = blk[:, :, INC]
        den = np.where(den != 0.0, den, 1.0)
        outp[k * CPC : (k + 1) * CPC] = (pool / den[:, :, None]).reshape(
            CPC, NHEAD * INC
        )
    return outp


def kernel(context_h, W_lin, b_lin, W_att, b_att, context_y, num_classes):
    global LAST_RESULT, LAST_PROFILE
    import os

    assert int(num_classes) == NCLS

    from concourse.bass_utils import run_bass_kernel_spmd

    # fixed oct capacity; bump tiles-per-oct only if the data demands it
    y = np.asarray(context_y).astype(np.int64)
    cnt = np.bincount(y, minlength=NCLS)
    gmax = 0
    for k in range(NCORES):
        c = cnt[k * CPC : (k + 1) * CPC]
        for o in range(NOCT):
            gmax = max(gmax, int(c[o * CPO : min(CPC, (o + 1) * CPO)].sum()))
    o_tiles = max(1, -(-gmax // 128))

    in_maps = _prep_inputs(
        context_h, W_lin, b_lin, W_att, b_att, context_y, o_tiles
    )
    if o_tiles not in _PROG_CACHE:
        _PROG_CACHE[o_tiles] = build_program(o_tiles=o_tiles)
    nc = _PROG_CACHE[o_tiles]
    core_ids = list(range(NCORES))
    res = run_bass_kernel_spmd(nc, in_maps, core_ids)
    LAST_RESULT = res
    outp = _unshard(res)  # materialize before any profiled re-run

    if os.environ.get("KERNEL_PROFILE") == "1":
        try:
            LAST_PROFILE = run_bass_kernel_spmd(
                nc, in_maps, core_ids, trace=True
            )
        except Exception:
            LAST_PROFILE = None

    return outp
